# revision 8
# baseline (speedup 1.0000x reference)
"""Trainium2 Bass kernel for nn_CausalAttentionKVCache (B=2, T=2048, D=1024, 16 heads).

Sharding: 8 cores = 2 batch-halves x 4 head-groups (4 heads each).
Two compiled SPMD programs (one per batch-half, phase constants differ mod 3),
dispatched concurrently on jax devices [0:4] and [4:8].

The module's reshape y.view(3,B,T,hs,nh) scrambles tokens: flat row
v = (c*B*T + b*T + t)//3 of y=[x@W+b] in column block j=(c*B*T+b*T+t)%3 holds
token t of tensor c (q/k/v). With a host-side column permutation of W
(W2[:, j*1024+h*64+d] = W[:, j*1024+d*16+h]) each head's 64 features are
contiguous and all three tensors share the same weight/bias blocks (WQK/BQK):
q/k/v differ only in which x-row window feeds the projection and the
residue->column-block map.

All matmul operands are bf16 (PSUM stays f32). Q^T, K^T and V^T are all
descrambled into token order by strided PSUM evictions (DVE), so attention
runs on contiguous 128-token chunks: the causal mask is a single 128-wide
affine_select on the diagonal chunk only. V^T is flipped to V[token, d] by
xbar DMA transposes (16x128 tiles, no PE cost) with a ones-column appended
for the softmax denominator. S^T = K^T.T@Q^T (k on partitions) -> exp on
ScalarE (scale=1/8 fused) -> PV re-oriented with P^T stationary:
ctx[q,65] += P^T[k,128q-block].T @ V[k,65], 65 cycles per 128x128 block.
Transpose-free epilogue: per-partition reciprocal of the denominator column,
scale, direct DMA out. Projections are split into 4 v-ranges so the first
attention window starts ~10us in, streaming the rest as fillers.
"""
import sys
import os

sys.path.insert(0, "/opt/trn_rl_repo")

import numpy as np

import concourse.bass as bass
import concourse.bacc as bacc
import concourse.mybir as mybir
import concourse.tile as tile

B, T, D, NH, HS = 2, 2048, 1024, 16, 64
NV = 684          # v-rows per (c, batch-half) slice
NCHUNK = 16       # k/v chunks of 128 tokens
QW = 512          # q window
BF16 = mybir.dt.bfloat16
F32 = mybir.dt.float32
VS = [(0, 172), (172, 344), (344, 516), (516, 684)]  # v-range splits

_CACHE = {}


def _phase(B2):
    """Compile-time residue/offset constants for batch-half B2."""
    cst = {}
    for c in range(3):
        u0 = c * B * T + B2 * T
        vstart = u0 // 3
        rc_of_jj, r0_of_jj = {}, {}
        for rc in range(3):
            jj = (u0 + rc) % 3
            rc_of_jj[jj] = rc
            r0_of_jj[jj] = (u0 + rc - jj) // 3 - vstart
        cst[c] = dict(u0=u0, vstart=vstart, rc=rc_of_jj, r0=r0_of_jj)
    return cst


def _build_program(B2, repeat=1):
    cst = _phase(B2)
    nc = bacc.Bacc("TRN2", target_bir_lowering=False, debug=False, num_devices=4)

    xtq_d = nc.dram_tensor("XTQ", [D, 768], BF16, kind="ExternalInput")
    xtk_d = nc.dram_tensor("XTK", [D, 768], BF16, kind="ExternalInput")
    xtv_d = nc.dram_tensor("XTV", [D, 768], BF16, kind="ExternalInput")
    wqk_d = nc.dram_tensor("WQK", [D, 768], BF16, kind="ExternalInput")
    bqk_d = nc.dram_tensor("BQK", [128, 6], F32, kind="ExternalInput")
    zr_d = nc.dram_tensor("ZR", [1, 260], BF16, kind="ExternalInput")
    out_d = nc.dram_tensor("OUT", [T, 256], F32, kind="ExternalOutput")

    xsrc = {0: xtq_d, 1: xtk_d, 2: xtv_d}

    with tile.TileContext(nc) as tc:
        with (
            tc.tile_pool(name="wpool", bufs=1) as wpool,
            tc.tile_pool(name="xpool", bufs=3) as xpool,
            tc.tile_pool(name="qkv", bufs=1) as qkvp,
            tc.tile_pool(name="ppool", bufs=6) as ppool,
            tc.tile_pool(name="opool", bufs=4) as opool,
            tc.tile_pool(name="rpool", bufs=4) as rpool,
        ):
            from contextlib import ExitStack
            wqk = wpool.tile([128, 8, 768], BF16)
            bqk = wpool.tile([128, 6], F32)
            zr = wpool.tile([1, 260], BF16)
            nc.sync.dma_start(bqk[:], bqk_d[:, :])
            nc.sync.dma_start(zr[:], zr_d[:, :])

            for _rep in range(repeat):
                proj_ctx = ExitStack()
                psqk = proj_ctx.enter_context(
                    tc.tile_pool(name="psqk", bufs=3, space="PSUM"))
                qt = qkvp.tile([128, 2, T], BF16, tag="qt")
                kt = qkvp.tile([128, 2, T], BF16, tag="kt")
                vt = qkvp.tile([128, 2, T], BF16, tag="vt")
                v_sb = qkvp.tile([128, NCHUNK, 4, 65], BF16, tag="v_sb")
                nc.vector.memset(v_sb[:, :, :, 64:65], 1.0)

                xts = {
                    si: xpool.tile([128, 8, 768], BF16, tag="xt",
                                   name=f"x{si}")
                    for si in range(3)
                }
                # DMA order: split-A columns of q, k, v first so the
                # projection pipeline starts early; the rest streams after.
                lo_a, hi_a = VS[0]
                for ic in range(8):
                    if _rep == 0:
                        nc.sync.dma_start(
                            wqk[:, ic, :],
                            wqk_d.rearrange("(c p) f -> p c f", p=128)[:, ic, :])
                    nc.sync.dma_start(
                        xts[0][:, ic, lo_a:hi_a],
                        xsrc[0].rearrange("(c p) v -> p c v", p=128)
                        [:, ic, lo_a:hi_a])
                for si in (1, 2):
                    for ic in range(8):
                        nc.sync.dma_start(
                            xts[si][:, ic, lo_a:hi_a],
                            xsrc[si].rearrange("(c p) v -> p c v", p=128)
                            [:, ic, lo_a:hi_a])
                for si in range(3):
                    for ic in range(8):
                        nc.sync.dma_start(
                            xts[si][:, ic, hi_a:768],
                            xsrc[si].rearrange("(c p) v -> p c v", p=128)
                            [:, ic, hi_a:768])

                # ---- projection emitter (q/k/v unified) ----
                def emit_proj(si, fc, k, pool=None, tag="psqk"):
                    jj, sub = fc // 2, fc % 2
                    rc = cst[si]["rc"][jj]
                    r0 = cst[si]["r0"][jj]
                    nrc = 683 if rc < 2 else 682
                    lo, hi = VS[k]
                    n = hi - lo
                    ps = (pool or psqk).tile([128, 172], F32, tag=tag,
                                             name="psp")
                    for ic in range(8):
                        nc.tensor.matmul(
                            ps[:, 0:n],
                            wqk[:, ic, fc * 128:(fc + 1) * 128],
                            xts[si][:, ic, lo:hi],
                            start=(ic == 0),
                            stop=(ic == 7),
                        )
                    vv0 = max(lo, r0)
                    vv1 = min(hi, r0 + nrc)
                    if vv1 <= vv0:
                        return
                    t0 = 3 * (vv0 - r0) + rc
                    t1 = min(t0 + 3 * (vv1 - vv0), T)
                    dst = (qt, kt, vt)[si]
                    nc.vector.tensor_scalar_add(
                        dst[:, sub, t0:t1:3],
                        ps[:, vv0 - lo: vv1 - lo],
                        bqk[:, fc: fc + 1],
                    )

                def emit_vtr(m):
                    for h in range(4):
                        fg, hr2 = h // 2, h % 2
                        nc.sync.dma_start_transpose(
                            v_sb[:, m, h, 0:64],
                            vt[hr2 * 64:(hr2 + 1) * 64, fg,
                               128 * m:128 * (m + 1)],
                        )

                # ---- attention emitters ----
                def emit_s_exp(hp, q0, m):
                    a = max(0, 128 * m - q0)
                    s_ps = pss.tile([128, 2 * QW], F32, tag="s", name="s_ps")
                    for hr in range(2):
                        pr = slice(hr * 64, hr * 64 + 64)
                        nc.tensor.matmul(
                            s_ps[:, hr * QW + a: (hr + 1) * QW],
                            kt[pr, hp, 128 * m: 128 * (m + 1)],
                            qt[pr, hp, q0 + a: q0 + QW],
                            start=True,
                            stop=True,
                            tile_position=(hr * 64, 0),
                        )
                    p_sb = ppool.tile([128, 2, QW], BF16, tag="p", name="p_sb")
                    s3 = s_ps[:].rearrange("p (h w) -> p h w", h=2)
                    nc.scalar.activation(
                        p_sb[:, :, a:QW],
                        s3[:, :, a:QW],
                        mybir.ActivationFunctionType.Exp,
                        scale=float(HS) ** -0.5,
                    )
                    if 128 * m >= q0:   # diagonal chunk: causal staircase
                        nc.gpsimd.affine_select(
                            out=p_sb[:, :, a:a + 128],
                            in_=p_sb[:, :, a:a + 128],
                            pattern=[[0, 2], [1, 128]],
                            compare_op=mybir.AluOpType.is_ge,
                            fill=0.0,
                            base=q0 + a - 128 * m,
                            channel_multiplier=-1,
                        )
                    return p_sb

                def emit_ctx_zero(ctx):
                    # matmul start=True zeroes the whole 2KB PSUM bank, so
                    # the 4 qb sub-regions cannot each carry their own
                    # start flag: zero the bank once (K=1 zero matmul) and
                    # accumulate everything with start=False.
                    for hr in range(2):
                        nc.tensor.matmul(
                            ctx[hr][:, :, :],
                            zr[0:1, 0:128],
                            zr[0:1, 0:260],
                            start=True,
                            stop=False,
                            skip_group_check=True,
                        )

                def emit_pv(hp, ctx, q0, m, p_sb):
                    a = max(0, 128 * m - q0)
                    for hr in range(2):
                        h_loc = 2 * hp + hr
                        for qb in range(a // 128, 4):
                            nc.tensor.matmul(
                                ctx[hr][:, qb, :],
                                p_sb[:, hr, qb * 128:(qb + 1) * 128],
                                v_sb[:, m, h_loc, :],
                                start=False,
                                stop=(m == q0 // 128 + qb),
                                skip_group_check=True,
                            )

                def make_epilogue(hp, q0, ctx):
                    def epi():
                        for qb in range(4):
                            o_sb = opool.tile([128, 2, 64], F32, tag="o",
                                              name="o_sb")
                            for hr in range(2):
                                rec = rpool.tile([128, 1], F32, tag="rec",
                                                 name="rec")
                                nc.vector.reciprocal(
                                    rec[:], ctx[hr][:, qb, 64:65])
                                nc.vector.tensor_scalar_mul(
                                    o_sb[:, hr, :], ctx[hr][:, qb, 0:64],
                                    rec[:])
                            nc.sync.dma_start(
                                out_d[q0 + qb * 128: q0 + (qb + 1) * 128,
                                      hp * 128:(hp + 1) * 128],
                                o_sb[:],
                            )
                    return epi

                # ---- emission schedule ----
                # lead-in: split-A projections for hp0 attention + V, then
                # the first four V chunks transposed
                for fc in (0, 2, 4):
                    emit_proj(0, fc, 0)
                for fc in (0, 2, 4):
                    emit_proj(1, fc, 0)
                for fc in range(6):
                    emit_proj(2, fc, 0)
                for m in range(4):
                    emit_vtr(m)
                proj_ctx.close()
                attn_ctx = ExitStack()
                pss = attn_ctx.enter_context(
                    tc.tile_pool(name="pss", bufs=2, space="PSUM"))
                psctx = attn_ctx.enter_context(
                    tc.tile_pool(name="psctx", bufs=2, space="PSUM"))

                def fp(si, fc, k):
                    return lambda: emit_proj(si, fc, k, pool=pss, tag="s")

                def ftr(m):
                    return lambda: emit_vtr(m)

                fillers = {
                    (0, 0): [fp(si, fc, 1) for si in (0, 1) for fc in (0, 2, 4)]
                            + [fp(2, fc, 1) for fc in range(6)]
                            + [ftr(m) for m in (4, 5, 6, 7)],
                    (0, 1): [fp(si, fc, 2) for si in (0, 1) for fc in (0, 2, 4)]
                            + [fp(2, fc, 2) for fc in range(6)]
                            + [ftr(m) for m in (8, 9, 10, 11)],
                    (0, 2): [fp(si, fc, 3) for si in (0, 1) for fc in (0, 2, 4)]
                            + [fp(2, fc, 3) for fc in range(6)]
                            + [ftr(m) for m in (12, 13, 14, 15)],
                    (0, 3): [fp(si, fc, 0) for si in (0, 1) for fc in (1, 3, 5)],
                    (1, 0): [fp(si, fc, 1) for si in (0, 1) for fc in (1, 3, 5)],
                    (1, 1): [fp(si, fc, 2) for si in (0, 1) for fc in (1, 3, 5)],
                    (1, 2): [fp(si, fc, 3) for si in (0, 1) for fc in (1, 3, 5)],
                }

                DEPTH = 3
                deferred_epi = None
                for hp in range(2):
                    for qi, q0 in enumerate(range(0, T, QW)):
                        nm = q0 // 128 + 4
                        fill = list(fillers.get((hp, qi), []))
                        ctx = [
                            psctx.tile([128, 4, 65], F32, tag=f"ctx{hr}",
                                       name=f"ctx{hr}")
                            for hr in range(2)
                        ]
                        emit_ctx_zero(ctx)
                        pend = []
                        for m in range(nm):
                            pend.append((m, emit_s_exp(hp, q0, m)))
                            if m == 4 and deferred_epi is not None:
                                deferred_epi()
                                deferred_epi = None
                            if fill:
                                fill.pop(0)()
                            if len(pend) > DEPTH:
                                m0, p0 = pend.pop(0)
                                emit_pv(hp, ctx, q0, m0, p0)
                        if deferred_epi is not None:
                            deferred_epi()
                            deferred_epi = None
                        while fill:
                            fill.pop(0)()
                        for m0, p0 in pend:
                            emit_pv(hp, ctx, q0, m0, p0)
                        deferred_epi = make_epilogue(hp, q0, ctx)
                deferred_epi()
                attn_ctx.close()

    nc.compile()
    return nc



# ---------------------------------------------------------------------------
# host-side data prep
# ---------------------------------------------------------------------------

def _perm_cols():
    perm = np.empty(3 * D, dtype=np.int64)
    for j in range(3):
        for h in range(NH):
            for d in range(HS):
                perm[j * D + h * HS + d] = j * D + d * NH + h
    return perm


def _host_dt():
    import ml_dtypes
    return ml_dtypes.bfloat16


def _core_inputs(xT, W2, b2, B2, HG):
    """xT/W2 already in the matmul host dtype; b2 f32."""
    bf16 = _host_dt()
    cst = _phase(B2)

    def xt_slice(c):
        vs = cst[c]["vstart"]
        sl = np.zeros((D, 768), dtype=bf16)
        lo, hi = max(0, vs), min(B * T, vs + 768)
        sl[:, lo - vs: hi - vs] = xT[:, lo:hi]
        return sl

    WQK = np.empty((D, 768), dtype=bf16)
    BQKf = np.empty(768, dtype=np.float32)
    for jj in range(3):
        src = jj * D + HG * 256
        WQK[:, jj * 256:(jj + 1) * 256] = W2[:, src:src + 256]
        BQKf[jj * 256:(jj + 1) * 256] = b2[src:src + 256]
    BQK = BQKf.reshape(6, 128).T.copy()  # [128, 6]: col fc, partition p

    return {
        "XTQ": xt_slice(0),
        "XTK": xt_slice(1),
        "XTV": xt_slice(2),
        "WQK": WQK,
        "BQK": np.ascontiguousarray(BQK),
        "ZR": np.zeros((1, 260), dtype=bf16),
    }


# ---------------------------------------------------------------------------
# concurrent two-program dispatch (4+4 cores)
# ---------------------------------------------------------------------------

def _sharded_fn(nc, dev_lo, dev_hi):
    import jax
    from jax.sharding import Mesh, PartitionSpec
    from jax.experimental.shard_map import shard_map
    from concourse import bass2jax
    from concourse.bass2jax import _bass_exec_p, install_neuronx_cc_hook

    install_neuronx_cc_hook()
    n_cores = dev_hi - dev_lo

    in_names, out_names, out_avals, zero_shapes = [], [], [], []
    partition_name = (
        nc.partition_id_tensor.name if nc.partition_id_tensor else None
    )
    for alloc in nc.m.functions[0].allocations:
        if not isinstance(alloc, mybir.MemoryLocationSet):
            continue
        name = alloc.memorylocations[0].name
        if alloc.kind == "ExternalInput":
            if name != partition_name:
                in_names.append(name)
        elif alloc.kind == "ExternalOutput":
            np_dt = mybir.dt.np(alloc.dtype)
            out_avals.append(
                jax.core.ShapedArray(tuple(alloc.tensor_shape), np_dt)
            )
            out_names.append(name)
            zero_shapes.append((tuple(alloc.tensor_shape), np_dt))
    n_params = len(in_names)
    all_in_names = list(in_names) + list(out_names)
    if partition_name is not None:
        all_in_names.append(partition_name)

    donate = tuple(range(n_params, n_params + len(out_names)))

    def _body(*args):
        operands = list(args)
        if partition_name is not None:
            operands.append(bass2jax.partition_id_tensor())
        outs = _bass_exec_p.bind(
            *operands,
            out_avals=tuple(out_avals),
            in_names=tuple(all_in_names),
            out_names=tuple(out_names),
            lowering_input_output_aliases=(),
            sim_require_finite=True,
            sim_require_nnan=True,
            nc=nc,
        )
        return tuple(outs)

    devices = jax.devices()[dev_lo:dev_hi]
    mesh = Mesh(np.asarray(devices), ("core",))
    in_specs = (PartitionSpec("core"),) * (n_params + len(out_names))
    out_specs = (PartitionSpec("core"),) * len(out_names)
    fn = jax.jit(
        shard_map(_body, mesh=mesh, in_specs=in_specs, out_specs=out_specs,
                  check_rep=False),
        donate_argnums=donate,
        keep_unused=True,
    )
    return fn, in_names, out_names, out_avals, zero_shapes, n_cores


def _concat_inputs(in_maps, in_names):
    return [
        np.concatenate([np.asarray(m[name]) for m in in_maps], axis=0)
        for name in in_names
    ]


def kernel(x, W_qkv, b_qkv):
    bf16 = _host_dt()
    x = np.asarray(x, dtype=np.float32)
    W_qkv = np.asarray(W_qkv, dtype=np.float32)
    b_qkv = np.asarray(b_qkv, dtype=np.float32)

    if "progs" not in _CACHE:
        _CACHE["progs"] = {
            B2: _build_program(B2, repeat=int(os.environ.get("KREPEAT", "1")))
            for B2 in range(2)
        }
        _CACHE["fns"] = {
            0: _sharded_fn(_CACHE["progs"][0], 0, 4),
            1: _sharded_fn(_CACHE["progs"][1], 4, 8),
        }

    perm = _perm_cols()
    W2 = W_qkv[:, perm].astype(bf16)
    b2 = b_qkv[perm]
    xT = np.ascontiguousarray(x.reshape(B * T, D).T).astype(bf16)

    results = {}
    pending = []
    for B2 in range(2):
        fn, in_names, out_names, out_avals, zero_shapes, n_cores = _CACHE["fns"][B2]
        in_maps = [_core_inputs(xT, W2, b2, B2, HG) for HG in range(4)]
        concat_in = _concat_inputs(in_maps, in_names)
        concat_zeros = [
            np.zeros((n_cores * s[0], *s[1:]), d) for (s, d) in zero_shapes
        ]
        out_arrs = fn(*concat_in, *concat_zeros)  # async dispatch
        pending.append((B2, out_names, out_avals, n_cores, out_arrs))

    out_full = np.zeros((B, T, D), dtype=np.float32)
    for B2, out_names, out_avals, n_cores, out_arrs in pending:
        per_core = np.asarray(out_arrs[0]).reshape(n_cores, T, 256)
        for HG in range(4):
            out_full[B2, :, HG * 256:(HG + 1) * 256] = per_core[HG]
    return out_full
